# revision 7
# baseline (speedup 1.0000x reference)
"""Trainium2 Bass kernel for OctahedralCavityProcessor.

Sharding: data-parallel over batch (B=8 -> 8 cores, zero collectives).
Each core processes one batch element b:
  phase A: cavity pooling  feat[k,c] = sum_p x[c,p] * mask_scaled[p,k]
           (PE transpose of x chunks + matmul accumulate into PSUM)
  phase B: per-cavity MLP + 14-token multi-head attention (tiny, on-chip)
  phase C: out[c,p] = x[c,p] + att_kc[. ,c] @ onehot[.,p]  (matmul + add)

Geometry-only quantities (mask, counts, nearest/onehot) and all weight
transposes are precomputed host-side in numpy; they do not depend on x.
"""

import numpy as np

import concourse.bass as bass
import concourse.tile as tile
from concourse import mybir
from concourse.bass_utils import run_bass_kernel_spmd
from concourse.vector_clock import ScopedClock, VectorClock
from contextlib import ExitStack

F32 = mybir.dt.float32

B, C, P, K, H = 8, 128, 100000, 14, 8
C2 = 2 * C
Dh = C // H
RADIUS = np.float32(0.5)

CHA = 128                     # phase A point-chunk (transpose width)
NA = (P + CHA - 1) // CHA     # 782 chunks, last has 32 points
XG = 16                       # x chunks loaded per big DMA in phase A
CHC = 512                     # phase C point-chunk
NCC = (P + CHC - 1) // CHC    # 196 chunks, last has 160 points


def _legalize_bir_waits(bir_json: bytes) -> bytes:
    """walrus here accepts at most ONE sync-wait command per instruction.
    Tile's scheduler may attach several.  Hoist the extras onto NoOp
    instructions inserted immediately before, on the same engine (the
    engine executes serially, so waiting one-at-a-time is equivalent)."""
    import json as _json

    d = _json.loads(bir_json)
    changed = False
    for fn in d.get("functions", []):
        for blk in fn.get("blocks", []):
            insts = blk.get("instructions", [])
            out = []
            for ins in insts:
                waits = (ins.get("sync_info") or {}).get("on_wait", [])
                if len(waits) > 1:
                    changed = True
                    for i, w in enumerate(waits[:-1]):
                        out.append({
                            "debug": ins.get("debug", 0),
                            "engine": ins["engine"],
                            "ins": [],
                            "name": f"{ins['name']}-wsplit{i}",
                            "opcode": "NoOp",
                            "outs": [],
                            "sync_info": {"on_update": [], "on_wait": [w]},
                            "text_hint": "wait_split",
                        })
                    ins["sync_info"]["on_wait"] = [waits[-1]]
                out.append(ins)
            blk["instructions"] = out
    if not changed:
        return bir_json
    return _json.dumps(d).encode()


def _install_wait_legalizer():
    import concourse.bass2jax as _b2j

    orig = _b2j.compile_bir_kernel
    if getattr(orig, "_wait_legalized", False):
        return

    def patched(bir_json, tmpdir, neff_name="file.neff"):
        return orig(_legalize_bir_waits(bir_json), tmpdir, neff_name=neff_name)

    patched._wait_legalized = True
    _b2j.compile_bir_kernel = patched


_install_wait_legalizer()


class SplitDrainTileContext(tile.TileContext):
    """The walrus build here only accepts ONE sync-wait command per
    instruction; stock TileContext puts every live sem wait on the tail
    Drain.  Split them across nop instructions instead."""

    def _drain_and_barrier(self, tick_clock, wait_clock):
        gc = tick_clock.global_clock
        n = len(gc)
        for i in range(n):
            if gc[i] <= 0:
                continue
            vec = [gc[j] if j == i else 0 for j in range(n)]
            nop = self.nc.sync.nop(nofuse=True, hint="tail_drain_split")
            wait_clock.add_sem_waits(nop.ins, ScopedClock({None: VectorClock(vec)}))
        self.nc.sync.drain()
        self.nc.all_engine_barrier()
        assert self.sems is not None
        popped = self.nc._tile_sem_poison_stack.pop()
        assert popped is self._sem_poison
        self.nc.clear_and_free_semaphores(list(self.sems.allocated().values()))
        self.nc.all_engine_barrier()


def build_program():
    nc = bass.Bass()

    x_d = nc.dram_tensor("x", [C, P], F32, kind="ExternalInput")
    maskA_d = nc.dram_tensor("maskA", [NA, CHA, K], F32, kind="ExternalInput")
    onehot_d = nc.dram_tensor("onehot", [K, P], F32, kind="ExternalInput")
    w1t_d = nc.dram_tensor("w1t", [K, C, C2], F32, kind="ExternalInput")
    w2t_d = nc.dram_tensor("w2t", [K, 2, C, C], F32, kind="ExternalInput")
    b1t_d = nc.dram_tensor("b1t", [C, 2 * K], F32, kind="ExternalInput")
    b2t_d = nc.dram_tensor("b2t", [C, K], F32, kind="ExternalInput")
    wq_d = nc.dram_tensor("wq", [C, C], F32, kind="ExternalInput")
    wk_d = nc.dram_tensor("wk", [C, C], F32, kind="ExternalInput")
    wv_d = nc.dram_tensor("wv", [C, C], F32, kind="ExternalInput")
    wo_d = nc.dram_tensor("wo", [Dh, H * C], F32, kind="ExternalInput")
    qb_d = nc.dram_tensor("qb", [Dh, H], F32, kind="ExternalInput")
    kb_d = nc.dram_tensor("kb", [Dh, H], F32, kind="ExternalInput")
    vb_d = nc.dram_tensor("vb", [K, C], F32, kind="ExternalInput")
    ob_d = nc.dram_tensor("ob", [C, 1], F32, kind="ExternalInput")
    ident_d = nc.dram_tensor("ident", [C, C], F32, kind="ExternalInput")
    out_d = nc.dram_tensor("out", [C, P], F32, kind="ExternalOutput")

    with SplitDrainTileContext(nc) as tc:
        with ExitStack() as octx:
            cpool = octx.enter_context(tc.tile_pool(name="consts", bufs=1))
            sums_pool = octx.enter_context(
                tc.tile_pool(name="sums_ps", bufs=1, space="PSUM")
            )

            ident_s = cpool.tile([C, C], F32, tag="ident")
            nc.sync.dma_start(ident_s[:], ident_d[:])

            # ---------------- phase A: cavity pooling ----------------
            sums_ps = sums_pool.tile([K, C], F32, tag="sums")
            with ExitStack() as actx:
                xg_pool = actx.enter_context(tc.tile_pool(name="xg", bufs=3))
                m_pool = actx.enter_context(tc.tile_pool(name="mA", bufs=6))
                xt_pool = actx.enter_context(tc.tile_pool(name="xt", bufs=6))
                tp_pool = actx.enter_context(
                    tc.tile_pool(name="tp", bufs=4, space="PSUM")
                )

                xg_t = None
                for c in range(NA):
                    g, j = divmod(c, XG)
                    col0 = c * CHA
                    pts = min(CHA, P - col0)
                    if j == 0:
                        g0 = g * XG * CHA
                        gw = min(XG * CHA, P - g0)
                        xg_t = xg_pool.tile([C, XG * CHA], F32, tag="xg")
                        nc.sync.dma_start(xg_t[:, :gw], x_d[:, g0:g0 + gw])
                    m_t = m_pool.tile([CHA, K], F32, tag="m")
                    nc.scalar.dma_start(m_t[:], maskA_d[c])
                    tp_t = tp_pool.tile([CHA, C], F32, tag="tp")
                    nc.tensor.transpose(
                        tp_t[:pts, :], xg_t[:, j * CHA:j * CHA + pts], ident_s[:]
                    )
                    xt_s = xt_pool.tile([CHA, C], F32, tag="xts")
                    nc.vector.tensor_copy(xt_s[:pts, :], tp_t[:pts, :])
                    nc.tensor.matmul(
                        sums_ps[:],
                        lhsT=m_t[:pts, :],
                        rhs=xt_s[:pts, :],
                        start=(c == 0),
                        stop=(c == NA - 1),
                    )

            # ---------------- phase B: MLP + attention ----------------
            with ExitStack() as bctx:
                wp = bctx.enter_context(tc.tile_pool(name="wp", bufs=3))
                hp = bctx.enter_context(tc.tile_pool(name="hp", bufs=2))
                sp = bctx.enter_context(tc.tile_pool(name="sp_ps", bufs=4, space="PSUM"))

                # feat^T [C, K]
                f_s = cpool.tile([K, C], F32, tag="f_s")
                nc.vector.tensor_copy(f_s[:], sums_ps[:])
                tpf = sp.tile([C, K], F32, tag="sps")
                nc.tensor.transpose(tpf[:], f_s[:], ident_s[:K, :K])
                featT = cpool.tile([C, K], F32, tag="featT")
                nc.vector.tensor_copy(featT[:], tpf[:])

                b1t_s = cpool.tile([C, 2 * K], F32, tag="b1t")
                nc.sync.dma_start(b1t_s[:], b1t_d[:])
                b2t_s = cpool.tile([C, K], F32, tag="b2t")
                nc.sync.dma_start(b2t_s[:], b2t_d[:])
                procT = cpool.tile([C, K], F32, tag="procT")

                for k in range(K):
                    w1_s = wp.tile([C, C2], F32, tag="w1")
                    nc.sync.dma_start(w1_s[:], w1t_d[k])
                    w2a_s = wp.tile([C, C], F32, tag="w2a")
                    nc.sync.dma_start(w2a_s[:], w2t_d[k, 0])
                    w2b_s = wp.tile([C, C], F32, tag="w2b")
                    nc.sync.dma_start(w2b_s[:], w2t_d[k, 1])

                    ph = sp.tile([C, 2], F32, tag="sps")
                    nc.tensor.matmul(ph[:, 0:1], lhsT=w1_s[:, 0:C],
                                     rhs=featT[:, k:k + 1], start=True, stop=False)
                    nc.tensor.matmul(ph[:, 1:2], lhsT=w1_s[:, C:C2],
                                     rhs=featT[:, k:k + 1], start=False, stop=True)
                    h_s = hp.tile([C, 2], F32, tag="h")
                    nc.scalar.activation(h_s[:, 0:1], ph[:, 0:1],
                                         mybir.ActivationFunctionType.Relu,
                                         bias=b1t_s[:, 2 * k:2 * k + 1])
                    nc.scalar.activation(h_s[:, 1:2], ph[:, 1:2],
                                         mybir.ActivationFunctionType.Relu,
                                         bias=b1t_s[:, 2 * k + 1:2 * k + 2])
                    pp = sp.tile([C, 1], F32, tag="sps")
                    nc.tensor.matmul(pp[:], lhsT=w2a_s[:], rhs=h_s[:, 0:1],
                                     start=True, stop=False)
                    nc.tensor.matmul(pp[:], lhsT=w2b_s[:], rhs=h_s[:, 1:2],
                                     start=False, stop=True)
                    nc.scalar.activation(procT[:, k:k + 1], pp[:],
                                         mybir.ActivationFunctionType.Tanh,
                                         bias=b2t_s[:, k:k + 1])

                # ---- attention over K=14 cavities ----
                wq_s = cpool.tile([C, C], F32, tag="wq")
                nc.sync.dma_start(wq_s[:], wq_d[:])
                wk_s = cpool.tile([C, C], F32, tag="wk")
                nc.sync.dma_start(wk_s[:], wk_d[:])
                wv_s = cpool.tile([C, C], F32, tag="wv")
                nc.sync.dma_start(wv_s[:], wv_d[:])
                wo_s = cpool.tile([Dh, H * C], F32, tag="wo")
                nc.sync.dma_start(wo_s[:], wo_d[:])
                qb_s = cpool.tile([Dh, H], F32, tag="qb")
                nc.sync.dma_start(qb_s[:], qb_d[:])
                kb_s = cpool.tile([Dh, H], F32, tag="kb")
                nc.sync.dma_start(kb_s[:], kb_d[:])
                vb_s = cpool.tile([K, C], F32, tag="vb")
                nc.sync.dma_start(vb_s[:], vb_d[:])
                ob_s = cpool.tile([C, 1], F32, tag="ob")
                nc.sync.dma_start(ob_s[:], ob_d[:])

                Id = mybir.ActivationFunctionType.Identity

                # q/k in head-blocked layout [Dh, H*K]: col (h,i), row d
                pq = sp.tile([Dh, H * K], F32, tag="sps")
                for h in range(H):
                    nc.tensor.matmul(pq[:, h * K:(h + 1) * K],
                                     lhsT=wq_s[:, h * Dh:(h + 1) * Dh],
                                     rhs=procT[:],
                                     start=(h == 0), stop=(h == H - 1))
                qh_s = cpool.tile([Dh, H * K], F32, tag="qT")
                for h in range(H):
                    nc.scalar.activation(qh_s[:, h * K:(h + 1) * K],
                                         pq[:, h * K:(h + 1) * K], Id,
                                         bias=qb_s[:, h:h + 1])

                pk = sp.tile([Dh, H * K], F32, tag="sps")
                for h in range(H):
                    nc.tensor.matmul(pk[:, h * K:(h + 1) * K],
                                     lhsT=wk_s[:, h * Dh:(h + 1) * Dh],
                                     rhs=procT[:],
                                     start=(h == 0), stop=(h == H - 1))
                kh_s = cpool.tile([Dh, H * K], F32, tag="kT")
                for h in range(H):
                    nc.scalar.activation(kh_s[:, h * K:(h + 1) * K],
                                         pk[:, h * K:(h + 1) * K], Id,
                                         bias=kb_s[:, h:h + 1])

                pv = sp.tile([K, C], F32, tag="sps")
                nc.tensor.matmul(pv[:], lhsT=procT[:], rhs=wv_s[:])
                v_s = cpool.tile([K, C], F32, tag="v")
                nc.vector.tensor_add(v_s[:], pv[:], vb_s[:])

                psc = sp.tile([K, H * K], F32, tag="sps")
                for h in range(H):
                    nc.tensor.matmul(
                        psc[:, h * K:(h + 1) * K],
                        lhsT=qh_s[:, h * K:(h + 1) * K],
                        rhs=kh_s[:, h * K:(h + 1) * K],
                        start=(h == 0),
                        stop=(h == H - 1),
                    )
                negmax = cpool.tile([K, H], F32, tag="negmax")
                nc.vector.tensor_reduce(
                    out=negmax[:],
                    in_=psc[:].rearrange("p (h j) -> p h j", j=K),
                    op=mybir.AluOpType.max,
                    axis=mybir.AxisListType.X,
                    negate=True,
                )
                esc = cpool.tile([K, H * K], F32, tag="esc")
                for h in range(H):
                    nc.scalar.activation(
                        esc[:, h * K:(h + 1) * K], psc[:, h * K:(h + 1) * K],
                        mybir.ActivationFunctionType.Exp,
                        bias=negmax[:, h:h + 1],
                    )
                ssum = cpool.tile([K, H], F32, tag="ssum")
                nc.vector.tensor_reduce(
                    out=ssum[:],
                    in_=esc[:].rearrange("p (h j) -> p h j", j=K),
                    op=mybir.AluOpType.add,
                    axis=mybir.AxisListType.X,
                )
                rinv = cpool.tile([K, H], F32, tag="rinv")
                nc.vector.reciprocal(rinv[:], ssum[:])
                for h in range(H):
                    nc.vector.tensor_scalar_mul(
                        esc[:, h * K:(h + 1) * K], esc[:, h * K:(h + 1) * K],
                        rinv[:, h:h + 1],
                    )

                pat = sp.tile([K, H * K], F32, tag="sps")
                for h in range(H):
                    nc.tensor.matmul(
                        pat[:, h * K:(h + 1) * K],
                        lhsT=esc[:, h * K:(h + 1) * K],
                        rhs=ident_s[:K, :K],
                        is_transpose=True,
                        start=(h == 0),
                        stop=(h == H - 1),
                    )
                at_s = cpool.tile([K, H * K], F32, tag="at")
                nc.vector.tensor_copy(at_s[:], pat[:])

                # o in head-blocked layout [Dh, H*K]
                po = sp.tile([Dh, H * K], F32, tag="sps")
                for h in range(H):
                    nc.tensor.matmul(
                        po[:, h * K:(h + 1) * K],
                        lhsT=v_s[:, h * Dh:(h + 1) * Dh],
                        rhs=at_s[:, h * K:(h + 1) * K],
                        start=(h == 0),
                        stop=(h == H - 1),
                    )
                o_s = cpool.tile([Dh, H * K], F32, tag="o")
                nc.vector.tensor_copy(o_s[:], po[:])

                # attT[e,i] = sum_h Wo[:, h-block] @ o_head_h  (accumulate)
                patt = sp.tile([C, K], F32, tag="sps")
                for h in range(H):
                    nc.tensor.matmul(patt[:],
                                     lhsT=wo_s[:, h * C:(h + 1) * C],
                                     rhs=o_s[:, h * K:(h + 1) * K],
                                     start=(h == 0), stop=(h == H - 1))
                attT_s = cpool.tile([C, K], F32, tag="attT")
                nc.scalar.activation(attT_s[:], patt[:], Id, bias=ob_s[:])

                pak = sp.tile([K, C], F32, tag="sps")
                nc.tensor.transpose(pak[:], attT_s[:], ident_s[:])
                ak_s = cpool.tile([K, C], F32, tag="ak")
                nc.vector.tensor_copy(ak_s[:], pak[:])

            # ---------------- phase C: gather-add ----------------
            with ExitStack() as cctx:
                xc_pool = cctx.enter_context(tc.tile_pool(name="xc", bufs=4))
                oh_pool = cctx.enter_context(tc.tile_pool(name="oh", bufs=4))
                oc_pool = cctx.enter_context(tc.tile_pool(name="oc", bufs=4))
                pc_pool = cctx.enter_context(
                    tc.tile_pool(name="pc", bufs=4, space="PSUM")
                )
                for j in range(NCC):
                    col0 = j * CHC
                    w = min(CHC, P - col0)
                    xc_t = xc_pool.tile([C, CHC], F32, tag="xc")
                    nc.sync.dma_start(xc_t[:, :w], x_d[:, col0:col0 + w])
                    oh_t = oh_pool.tile([K, CHC], F32, tag="oh")
                    nc.scalar.dma_start(oh_t[:, :w], onehot_d[:, col0:col0 + w])
                    pc_t = pc_pool.tile([C, CHC], F32, tag="pc")
                    nc.tensor.matmul(pc_t[:, :w], lhsT=ak_s[:], rhs=oh_t[:, :w],
                                     start=True, stop=True)
                    oc_t = oc_pool.tile([C, CHC], F32, tag="ocd")
                    nc.vector.tensor_add(oc_t[:, :w], pc_t[:, :w], xc_t[:, :w])
                    nc.sync.dma_start(out_d[:, col0:col0 + w], oc_t[:, :w])

    return nc


def prep_host(points, cavities, w1, b1, w2, b2, in_w, in_b, out_w, out_b):
    """Geometry + weight preprocessing (pure numpy, no x dependence)."""
    points = np.asarray(points, np.float32)
    cavities = np.asarray(cavities, np.float32)
    d = np.sqrt(
        ((points[None, :, :] - cavities[:, None, :]) ** 2).sum(-1, dtype=np.float32)
    ).astype(np.float32)                                   # [K, P]
    mask = (d < RADIUS).astype(np.float32)                 # [K, P]
    counts = mask.sum(axis=1, dtype=np.float32)            # [K]
    inv = np.where(counts > 0, 1.0 / np.maximum(counts, 1.0), 0.0).astype(np.float32)
    maskA = np.zeros((NA * CHA, K), np.float32)
    maskA[:P] = (mask.T * inv[None, :]).astype(np.float32)
    maskA = maskA.reshape(NA, CHA, K)

    nearest = np.argmin(d, axis=0)                         # [P]
    onehot = np.zeros((K, P), np.float32)
    onehot[nearest, np.arange(P)] = 1.0

    w1 = np.asarray(w1, np.float32)
    w2 = np.asarray(w2, np.float32)
    scale = np.float32(1.0 / np.sqrt(Dh))
    fp = {
        "maskA": np.ascontiguousarray(maskA),
        "onehot": np.ascontiguousarray(onehot),
        "w1t": np.ascontiguousarray(w1.transpose(0, 2, 1)),          # [K, C, 2C]
        "w2t": np.ascontiguousarray(
            w2.transpose(0, 2, 1).reshape(K, 2, C, C)),              # [K, 2, C, C]
        "b1t": np.ascontiguousarray(
            np.asarray(b1, np.float32).reshape(K, 2, C).transpose(2, 0, 1)
            .reshape(C, 2 * K)),
        "b2t": np.ascontiguousarray(np.asarray(b2, np.float32).T),   # [C, K]
        "wq": np.ascontiguousarray(
            np.asarray(in_w, np.float32)[0:C].T * scale),
        "wk": np.ascontiguousarray(np.asarray(in_w, np.float32)[C:2 * C].T),
        "wv": np.ascontiguousarray(np.asarray(in_w, np.float32)[2 * C:3 * C].T),
        # wo_heads[d, h*C+e] = out_w[e, h*Dh+d]
        "wo": np.ascontiguousarray(
            np.asarray(out_w, np.float32).reshape(C, H, Dh)
            .transpose(2, 1, 0).reshape(Dh, H * C)),
        # head-blocked biases [Dh, H]
        "qb": np.ascontiguousarray(
            (np.asarray(in_b, np.float32)[0:C] * scale).reshape(H, Dh).T),
        "kb": np.ascontiguousarray(
            np.asarray(in_b, np.float32)[C:2 * C].reshape(H, Dh).T),
        "vb": np.ascontiguousarray(
            np.tile(np.asarray(in_b, np.float32)[2 * C:3 * C], (K, 1))),
        "ob": np.ascontiguousarray(np.asarray(out_b, np.float32).reshape(C, 1)),
        "ident": np.eye(C, dtype=np.float32),
    }
    return fp


_PROGRAM = None


def kernel(x, points, cavities, w1, b1, w2, b2, in_w, in_b, out_w, out_b):
    global _PROGRAM
    x = np.asarray(x, np.float32)
    fp = prep_host(points, cavities, w1, b1, w2, b2, in_w, in_b, out_w, out_b)
    if _PROGRAM is None:
        _PROGRAM = build_program()
    nc = _PROGRAM
    in_maps = [dict(fp, x=np.ascontiguousarray(x[b])) for b in range(B)]
    res = run_bass_kernel_spmd(nc, in_maps, list(range(B)))
    out = np.stack([res.results[b]["out"] for b in range(B)], axis=0)
    return out.astype(np.float32)
